# revision 58
# baseline (speedup 1.0000x reference)
"""Trainium2 Bass kernel for nn_ChannelAttention (squeeze-excite).

Reference computation:
    s = mean(x, axis=(H, W))                    # [B, C]   global avg pool
    h = relu(bn1(s @ w1))                       # [B, Cr]  Cr = 16
    o = bn2(h @ w2)                             # [B, C]
    return o[:, None, None, :]                  # [B, 1, 1, C]

Strategy (data-parallel over batch, 8 cores x 8 samples; measured ~93-100 us
per core vs a ~71 us HBM roofline):
  - Each core streams its 8 samples as 4 sample-PAIR tiles [128, 12544]
    (6.4 MB contiguous per HWDGE DMA on the sync ring; 49 rows/partition,
    sample boundary exactly at partition 64). Big 128-partition DMAs on one
    ring measured ~400-460 GB/s vs ~235 GB/s for 112-partition per-sample
    tiles. The last pair is split into 3 chunk-DMAs so tail consumption
    starts before the final bytes land.
  - All 11 parameter tensors are packed host-side into ONE [128, 1316]
    input ("params") moved by a single HWDGE DMA on the scalar ring: tiny
    separate DMAs either steal HWDGE sem lanes from the x stream (each
    lane reuse stalls the next x DMA ~3 us on completion receipt) or, on
    SWDGE, complete pathologically late under stream saturation.
  - Squeeze: f32 matmul is 2 half-speed HW passes (~858 ns per [128,512]
    slice), so PE alone cannot keep up with the stream; the 24.5 column
    slices per pair are split PE/DVE. PE reduces its slices with an M=33
    pair-indicator lhsT (col 0 -> partitions 0..63 -> PSUM row 0, col 32
    -> partitions 64..127 -> PSUM row 32, satisfying 32-aligned partition
    bases), accumulating into a per-pair PSUM bank. DVE pre-reduces the
    remaining slices with a chain of f32 tensor_adds ((512+151)/0.96
    ~ 690 ns each, 1x mode), folded in by one extra PE matmul.
  - Per-sample [1,512] sums are copied to SBUF (ScalarE), folded to
    [1,256] (DVE), then gathered into an sT layout [128ch, 8samples] x2
    via K=1 one-hot matmuls (doubling as the transpose for the MLP).
  - Excite MLP on PE: g1[16,8] = w1.T @ sT (K=256 split in 2), BN1+ReLU
    as a single ScalarE activation (per-partition scale/bias APs, with
    the 1/HW mean scale folded into BN1's scale), o[8,256] = h.T @ w2,
    BN2 applied with parameters pre-replicated to [8,256] in the pack.
  - BN prep math is emitted after stage 1 so the in-order DVE stream
    never stalls on the params DMA mid-stream.
"""

import sys

if "/opt/trn_rl_repo" not in sys.path:
    sys.path.insert(0, "/opt/trn_rl_repo")

import numpy as np

B, H, W, C = 64, 56, 56, 256
CR = 16
NCORES = 8
BL = B // NCORES  # samples per core
HWP = H * W  # 3136 spatial positions
NPAIR = BL // 2  # 4 sample-pairs per core, one DMA each
PFD = 2 * HWP * C // 128  # 12544 free-dim elements per partition (49 rows)
NSL = PFD // 512  # 24 full 512-wide column slices (+ one 256 tail)
PW = 1316  # packed parameter tensor width (see _pack_params)
EPS = 1e-3

_CACHE: dict = {}


def _build_nc():
    import concourse.bass as bass
    import concourse.tile as tile
    from concourse import bacc, mybir
    from contextlib import ExitStack

    f32 = mybir.dt.float32
    AF = mybir.ActivationFunctionType

    nc = bacc.Bacc("TRN2", target_bir_lowering=False, debug=False)

    x_d = nc.dram_tensor("x", [NPAIR, 128, PFD], f32, kind="ExternalInput")
    par_d = nc.dram_tensor("params", [128, PW], f32, kind="ExternalInput")
    out_d = nc.dram_tensor("out", [BL, C], f32, kind="ExternalOutput")

    with ExitStack() as ctx:
        tc = ctx.enter_context(tile.TileContext(nc))
        xp = ctx.enter_context(tc.tile_pool(name="xp", bufs=4))
        pp = ctx.enter_context(tc.tile_pool(name="pp", bufs=1))
        accp = ctx.enter_context(tc.tile_pool(name="accp", bufs=4, space="PSUM"))
        mlpp = ctx.enter_context(tc.tile_pool(name="mlpp", bufs=1, space="PSUM"))

        # ---- x stream first: the sync HWDGE ring runs FIFO, so the big
        # DMAs are issued before anything else queues on it. The last pair
        # is split in two halves so its consumption can start ~8 us before
        # the final bytes land (shorter kernel tail).
        xts = []
        for q in range(NPAIR):
            xt = xp.tile([128, PFD], f32, tag="xt", name=f"xt{q}", bufs=3)
            if q < NPAIR - 1:
                nc.sync.dma_start(xt, x_d[q])
            else:
                nc.sync.dma_start(xt[:, 0:6144], x_d[q][:, 0:6144])
                nc.sync.dma_start(xt[:, 6144:10240], x_d[q][:, 6144:10240])
                nc.sync.dma_start(xt[:, 10240:PFD], x_d[q][:, 10240:PFD])
            xts.append(xt)

        # ---- constants / parameters (all overlap with the main stream) ----
        # pair indicator, M=33 so the two samples' sums land on the
        # 32-aligned PSUM partitions {0, 32} (compute-engine APs require
        # 32-aligned partition bases): col 0 selects partitions 0..63
        # (first sample of the pair), col 32 selects 64..127 (second).
        po = pp.tile([128, 33], f32, tag="po", name="po")
        nc.vector.memset(po, 0.0)
        nc.vector.memset(po[0:64, 0:1], 1.0)
        nc.vector.memset(po[64:128, 32:33], 1.0)

        # one-hot bank: oh[p, b, j] = (b == j), identical on every partition
        oh = pp.tile([128, BL, BL], f32, tag="oh", name="oh")
        nc.vector.memset(oh, 0.0)
        for b in range(BL):
            nc.vector.memset(oh[:, b, b : b + 1], 1.0)

        # ---- stage 1: squeeze (global sum over H*W per sample/channel) ----
        # acc_sb[32j, q, :]: raw [1, 512] channel sums for sample 2q + j
        acc_sb = pp.tile([128, NPAIR, 512], f32, tag="acc_sb", name="acc_sb")
        # s_sb[32j, q, :]: folded [1, 256] sums
        s_sb = pp.tile([128, NPAIR, C], f32, tag="s_sb", name="s_sb")
        # sT[c, b] per channel half -> MLP rhs
        sT0 = mlpp.tile([128, BL], f32, tag="sT0", name="sT0")
        sT1 = mlpp.tile([128, BL], f32, tag="sT1", name="sT1")

        # Work split inside a pair tile (25088 columns = 24.5 x 512):
        # PE reduces a few 512-slices directly with the pair-indicator
        # lhsT (f32 matmul = 2 HW passes, ~858 ns/slice) plus the 256
        # tail; DVE pre-reduces the bulk with 2048-wide tensor_adds
        # (~2.3 us each, amortizing the ~151-cycle per-op overhead),
        # and PE folds each 512 column block of the DVE partial.
        # Per-pair segments: (pe_slices, dve_block_cols, dve_width, tail)
        # so the split last pair consumes each chunk independently.
        # Per-pair segments: (pe_slice_list, (dve_block_cols, width), tail).
        # DVE pre-reduces 512-wide blocks with a chain of tensor_adds
        # (~690 ns each); PE folds each 512-col piece of the partial. The
        # last pair is the kernel tail: small PE share (PE runs cold
        # there), chunks consumed concurrently as each lands.
        def b512(ks):
            return ([k * 512 for k in ks], 512)

        segs = {
            q: [(list(range(0, 8)), b512(range(8, NSL)), True)]
            for q in range(NPAIR - 1)
        }
        segs[NPAIR - 1] = [
            ([0, 1, 2], b512(range(3, 12)), False),
            ([12, 13], b512(range(14, 20)), False),
            ([20], b512(range(21, NSL)), True),
        ]

        def emit_fold(q):
            # fold the [1,512] sums to [1,256]; emitted one pair late so
            # the in-order DVE stream never stalls on the ACT copy -> PE
            # fold dependency mid-stream
            for j in range(2):
                pb = 32 * j
                nc.vector.tensor_add(
                    s_sb[pb : pb + 1, q, :],
                    acc_sb[pb : pb + 1, q, 0:C],
                    acc_sb[pb : pb + 1, q, C : 2 * C],
                )

        ndve = 0
        for q in range(NPAIR):
            xt = xts[q]
            acc = accp.tile([128, 512], f32, tag="acc", name=f"acc{q}")
            first = True
            for si, (pe_ks, (dcols, dw), has_tail) in enumerate(segs[q]):
                last_seg = si == len(segs[q]) - 1
                for k in pe_ks:
                    nc.tensor.matmul(
                        acc[0:33, :],
                        po,
                        xt[:, k * 512 : (k + 1) * 512],
                        start=first,
                        stop=False,
                    )
                    first = False
                if has_tail:
                    # 256-wide tail column slice
                    nc.tensor.matmul(
                        acc[0:33, 0:256],
                        po,
                        xt[:, NSL * 512 :],
                        start=False,
                        stop=False,
                    )
                dve_acc = pp.tile(
                    [128, 512], f32, tag="dve_acc", name=f"dve{ndve}", bufs=4
                )
                ndve += 1
                nc.vector.tensor_add(
                    dve_acc[:, 0:dw],
                    xt[:, dcols[0] : dcols[0] + dw],
                    xt[:, dcols[1] : dcols[1] + dw],
                )
                for c0 in dcols[2:]:
                    nc.vector.tensor_add(
                        dve_acc[:, 0:dw], dve_acc[:, 0:dw], xt[:, c0 : c0 + dw]
                    )
                for fc in range(0, dw, 512):
                    nc.tensor.matmul(
                        acc[0:33, :],
                        po,
                        dve_acc[:, fc : fc + 512],
                        start=False,
                        stop=last_seg and (fc + 512 >= dw),
                    )

            # one wide copy grabs both samples' PSUM rows (0 and 32)
            nc.scalar.copy(acc_sb[0:33, q, :], acc[0:33, :])
            if q >= 1:
                emit_fold(q - 1)
        emit_fold(NPAIR - 1)

        # gather all pairs' sums into the sT layout (also the transpose);
        # emitted after the whole stream so no stage-1 PE work ever queues
        # behind a gather's cross-engine dependency
        for h, sT in enumerate((sT0, sT1)):
            for b in range(BL):
                q, pb = b // 2, 32 * (b % 2)
                nc.tensor.matmul(
                    sT[:, 0:BL],
                    s_sb[pb : pb + 1, q, h * 128 : (h + 1) * 128],
                    oh[pb : pb + 1, b, :],
                    start=(b == 0),
                    stop=(b == BL - 1),
                )

        # All parameters arrive pre-packed (host-side) in one [128, PW]
        # tensor via a single HWDGE DMA on the scalar ring — tiny separate
        # param DMAs either steal HWDGE sem lanes from the x stream or, on
        # SWDGE, complete pathologically late under stream saturation.
        pt = pp.tile([128, PW], f32, tag="pt", name="pt")
        nc.scalar.dma_start(pt, par_d[:, :])
        w1a = pt[:, 0:CR]
        w1b = pt[:, CR : 2 * CR]
        w2t = pt[0:CR, 32 : 32 + C]
        ga1 = pt[0:CR, 288:289]
        be1 = pt[0:CR, 289:290]
        mu1 = pt[0:CR, 290:291]
        va1 = pt[0:CR, 291:292]
        # BN2 params: gamma2/var2 replicated on rows 0..15 (to scale w2's
        # columns), and all four replicated once on row 32 (to build the
        # bias row of the augmented K=33 second matmul)
        ga2 = pt[0:CR, 292 : 292 + C]
        va2 = pt[0:CR, 548 : 548 + C]
        ga2r = pt[32:33, 292 : 292 + C]
        va2r = pt[32:33, 548 : 548 + C]
        be2r = pt[32:33, 804 : 804 + C]
        mu2r = pt[32:33, 1060 : 1060 + C]

        # scale1 = gamma1 / sqrt(var1 + eps) / HW, bias1 = beta1 - mean1 * k1
        # (route activation deps through a single engine: the Activation
        # instruction encoding only has room for one sync wait when bias
        # is an AP, so both of its inputs must come from the same sem)
        eps1 = pp.tile([CR, 1], f32, tag="eps1", name="eps1")
        nc.vector.memset(eps1, EPS)
        va1c = pp.tile([CR, 1], f32, tag="va1c", name="va1c")
        nc.vector.tensor_copy(va1c, va1)
        srt1 = pp.tile([CR, 1], f32, tag="srt1", name="srt1")
        nc.scalar.activation(srt1, va1c, AF.Sqrt, bias=eps1)
        rst1 = pp.tile([CR, 1], f32, tag="rst1", name="rst1")
        nc.vector.reciprocal(rst1, srt1)
        k1 = pp.tile([CR, 1], f32, tag="k1", name="k1")
        nc.vector.tensor_mul(k1, ga1, rst1)
        sc1 = pp.tile([CR, 1], f32, tag="sc1", name="sc1")
        nc.scalar.mul(sc1, k1, 1.0 / HWP)
        tm1 = pp.tile([CR, 1], f32, tag="tm1", name="tm1")
        nc.vector.tensor_mul(tm1, mu1, k1)
        bi1 = pp.tile([CR, 1], f32, tag="bi1", name="bi1")
        nc.vector.tensor_sub(bi1, be1, tm1)

        # BN2 is folded entirely into the second MLP matmul: the augmented
        # operand w2bi holds w2 * k2 on rows 0..15 and the BN2 bias vector
        # on row 32 (32-aligned); rows 16..31 are zeroed. h_ext gets a ones
        # row at 32, so o = h_ext.T @ w2bi computes bn2(h @ w2) directly.
        w2bi = pp.tile([33, C], f32, tag="w2bi", name="w2bi")
        nc.vector.memset(w2bi, 0.0)
        eps2 = pp.tile([33, 1], f32, tag="eps2", name="eps2")
        nc.vector.memset(eps2, EPS)
        va2c = pp.tile([CR, C], f32, tag="va2c", name="va2c")
        nc.vector.tensor_copy(va2c, va2)
        srt2 = pp.tile([CR, C], f32, tag="srt2", name="srt2")
        nc.scalar.activation(srt2, va2c, AF.Sqrt, bias=eps2[0:CR])
        rst2 = pp.tile([CR, C], f32, tag="rst2", name="rst2")
        nc.vector.reciprocal(rst2, srt2)
        k2 = pp.tile([CR, C], f32, tag="k2", name="k2")
        nc.vector.tensor_mul(k2, ga2, rst2)
        nc.vector.tensor_mul(w2bi[0:CR, :], w2t, k2)
        # bias row at partition 32
        va2rc = pp.tile([33, C], f32, tag="va2rc", name="va2rc")
        nc.vector.tensor_copy(va2rc[32:33, :], va2r)
        srt2r = pp.tile([33, C], f32, tag="srt2r", name="srt2r")
        nc.scalar.activation(srt2r[32:33, :], va2rc[32:33, :], AF.Sqrt, bias=eps2[32:33])
        rst2r = pp.tile([33, C], f32, tag="rst2r", name="rst2r")
        nc.vector.reciprocal(rst2r[32:33, :], srt2r[32:33, :])
        k2r = pp.tile([33, C], f32, tag="k2r", name="k2r")
        nc.vector.tensor_mul(k2r[32:33, :], ga2r, rst2r[32:33, :])
        tm2r = pp.tile([33, C], f32, tag="tm2r", name="tm2r")
        nc.vector.tensor_mul(tm2r[32:33, :], mu2r, k2r[32:33, :])
        nc.vector.tensor_sub(w2bi[32:33, :], be2r, tm2r[32:33, :])

        # ---- stage 2: excite MLP ----
        sT0s = pp.tile([128, BL], f32, tag="sT0s", name="sT0s")
        nc.scalar.copy(sT0s, sT0)
        sT1s = pp.tile([128, BL], f32, tag="sT1s", name="sT1s")
        nc.vector.tensor_copy(sT1s, sT1)

        g1p = mlpp.tile([CR, BL], f32, tag="g1p", name="g1p")
        nc.tensor.matmul(g1p, w1a, sT0s, start=True, stop=False)
        nc.tensor.matmul(g1p, w1b, sT1s, start=False, stop=True)

        # h = relu(g1 * scale1 + bias1)  (BN1 + mean scale + relu in one op).
        # bi1 comes from DVE; copy it through ACT so the Relu activation's
        # only cross-engine wait is on the PE matmul result. h_ext rows
        # 16..31 are zero and row 32 is ones (the BN2 bias row selector).
        bi1c = pp.tile([CR, 1], f32, tag="bi1c", name="bi1c")
        nc.scalar.copy(bi1c, bi1)
        sc1c = pp.tile([CR, 1], f32, tag="sc1c", name="sc1c")
        nc.scalar.copy(sc1c, sc1)
        h_ext = pp.tile([33, BL], f32, tag="h_ext", name="h_ext")
        nc.vector.memset(h_ext, 0.0)
        nc.vector.memset(h_ext[32:33, :], 1.0)
        nc.scalar.activation(h_ext[0:CR, :], g1p, AF.Relu, bias=bi1c, scale=sc1c)

        o_p = mlpp.tile([BL, C], f32, tag="o_p", name="o_p")
        nc.tensor.matmul(o_p, h_ext[0:33, 0:BL], w2bi[0:33, :], start=True, stop=True)

        ofin = pp.tile([BL, C], f32, tag="ofin", name="ofin")
        nc.vector.tensor_copy(ofin, o_p)
        nc.sync.dma_start(out_d[:, :], ofin)

    nc.compile()
    return nc


def _get_nc():
    if "nc" not in _CACHE:
        _CACHE["nc"] = _build_nc()
    return _CACHE["nc"]


def _pack_params(inputs):
    def g(k):
        return np.asarray(inputs[k], dtype=np.float32)

    p = np.zeros((128, PW), np.float32)
    w1 = g("w1")
    p[:, 0:CR] = w1[0:128]
    p[:, CR : 2 * CR] = w1[128:256]
    p[0:CR, 32 : 32 + C] = g("w2")
    p[0:CR, 288] = g("gamma1")
    p[0:CR, 289] = g("beta1")
    p[0:CR, 290] = g("mean1")
    p[0:CR, 291] = g("var1")
    p[0:CR, 292 : 292 + C] = g("gamma2")[None, :]
    p[0:CR, 548 : 548 + C] = g("var2")[None, :]
    p[32, 292 : 292 + C] = g("gamma2")
    p[32, 548 : 548 + C] = g("var2")
    p[32, 804 : 804 + C] = g("beta2")
    p[32, 1060 : 1060 + C] = g("mean2")
    return p


def _in_maps(inputs):
    x = np.ascontiguousarray(np.asarray(inputs["x"], dtype=np.float32))
    params = _pack_params(inputs)
    maps = []
    for c in range(NCORES):
        shard = np.ascontiguousarray(x[c * BL : (c + 1) * BL]).reshape(NPAIR, 128, PFD)
        maps.append({"x": shard, "params": params})
    return maps


def _run(inputs, trace=False):
    from concourse.bass_utils import run_bass_kernel_spmd

    nc = _get_nc()
    res = run_bass_kernel_spmd(
        nc, _in_maps(inputs), core_ids=list(range(NCORES)), trace=trace
    )
    out = np.concatenate([res.results[c]["out"] for c in range(NCORES)], axis=0)
    return out.reshape(B, 1, 1, C).astype(np.float32), res


def kernel(**inputs) -> np.ndarray:
    out, _ = _run(inputs, trace=False)
    return out


def kernel_traced(**inputs):
    """Returns (out, BassKernelResults) with NTFF profiling enabled."""
    return _run(inputs, trace=True)


def bench(inputs, iters=30, warmup=5):
    """Time the per-step NEFF execution with device-resident inputs.

    Returns (out_full, per_call_seconds_list). Inputs are device_put once;
    each timed call only dispatches the compiled executable, so steady-state
    per-call wall time ~= max-core NEFF exec + dispatch overhead.
    """
    import time
    import jax
    import jax.numpy as jnp
    from jax.sharding import Mesh, PartitionSpec, NamedSharding
    from jax.experimental.shard_map import shard_map
    from concourse import bass2jax, mybir

    bass2jax.install_neuronx_cc_hook()
    nc = _get_nc()

    partition_name = nc.partition_id_tensor.name if nc.partition_id_tensor else None
    in_names, out_names, out_avals = [], [], []
    for alloc in nc.m.functions[0].allocations:
        if not isinstance(alloc, mybir.MemoryLocationSet):
            continue
        name = alloc.memorylocations[0].name
        if alloc.kind == "ExternalInput":
            if name != partition_name:
                in_names.append(name)
        elif alloc.kind == "ExternalOutput":
            out_names.append(name)
            out_avals.append(
                jax.core.ShapedArray(tuple(alloc.tensor_shape), mybir.dt.np(alloc.dtype))
            )
    all_in_names = in_names + out_names
    if partition_name is not None:
        all_in_names = all_in_names + [partition_name]

    def _body(*operands):
        operands = list(operands)
        if partition_name is not None:
            operands.append(bass2jax.partition_id_tensor())
        outs = bass2jax._bass_exec_p.bind(
            *operands,
            out_avals=tuple(out_avals),
            in_names=tuple(all_in_names),
            out_names=tuple(out_names),
            lowering_input_output_aliases=(),
            sim_require_finite=True,
            sim_require_nnan=True,
            nc=nc,
        )
        return tuple(outs)

    devices = jax.devices()[:NCORES]
    mesh = Mesh(np.asarray(devices), ("core",))
    spec = PartitionSpec("core")
    maps = _in_maps(inputs)
    concat = [
        np.concatenate([maps[c][n] for c in range(NCORES)], axis=0) for n in in_names
    ]
    concat += [
        np.zeros((NCORES * a.shape[0], *a.shape[1:]), a.dtype) for a in out_avals
    ]
    sharding = NamedSharding(mesh, spec)
    dev_in = [jax.device_put(a, sharding) for a in concat]

    fn = jax.jit(
        shard_map(
            _body,
            mesh=mesh,
            in_specs=(spec,) * len(concat),
            out_specs=(spec,) * len(out_names),
            check_rep=False,
        )
    )

    for _ in range(warmup):
        outs = fn(*dev_in)
    jax.block_until_ready(outs)

    times = []
    for _ in range(iters):
        t0 = time.perf_counter()
        outs = fn(*dev_in)
        jax.block_until_ready(outs)
        times.append(time.perf_counter() - t0)

    oidx = out_names.index("out")
    o = np.asarray(outs[oidx]).reshape(NCORES, BL, C).reshape(B, C)
    return o.reshape(B, 1, 1, C).astype(np.float32), times
